# revision 6
# baseline (speedup 1.0000x reference)
"""Trainium2 Bass kernel: per-batch segment-mean pooling + 3-layer MLP.

Reference computation (B=64, T=512, H=768, S=128):
  pooled[b,s,:] = mean over t of hidden[b,t,:] where statements_ids[b,t]==s
  x = gelu(pooled @ w1 + b1); x = gelu(x @ w2 + b2)
  out[b,s] = sigmoid(x @ w3 + b3)

Distribution: data-parallel over batch across 8 NeuronCores (8 batches per
core); MLP weights replicated.

Per-core algorithm:
  - Host precomputes the count-normalized one-hot MT[t,s] = inv[s]*(sid[t]==s)
    (counts depend only on statements_ids) and ships it in fp16, along with
    fp16 hidden and weights.  fp16 keeps the PE at 1 cycle/row for any moving
    size and halves HBM traffic vs fp32; 10 mantissa bits keep the rel err
    ~1e-3 (tolerance 2e-2).
  - pooled^T tiles directly: matmul(lhsT=hidden[t,h-tile], rhs=MT[t,s])
    -> psum [128h, 128s], accumulated over the 4 t-tiles.  No PE transposes,
    no on-device one-hot build, no normalization chain: the psum already
    holds mean-pooled values in the [h, s] orientation the MLP wants.
  - psum -> SBUF copies (fp16) on DVE (GPSIMD can't read PSUM on TRN2).
  - MLP batched over all 8 local batches: rows = 8*128 = 1024 moving dim,
    weights stationary; gelu + bias fused on ACT.
  - sigmoid(z) = 0.5 + 0.5*tanh(z/2): tanh lives in the same ACT function
    table as gelu, so the 8 x 1.28us ACT_TABLE_LOAD thrash of alternating
    gelu/sigmoid tables disappears.  w3/b3 are pre-scaled by 0.5 on the
    host; the final 0.5*t+0.5 affine runs on the otherwise-idle DVE.
  - DMA issue (~600ns per descriptor on an engine's HWDGE queue) is split:
    weights on the ACT queue, the hidden/MT stream on sync, so neither
    stream's issue latency delays the other's arrival.
"""

import os
import sys

sys.path.insert(0, "/opt/trn_rl_repo")

import numpy as np

import concourse.bass as bass
import concourse.mybir as mybir
import concourse.tile as tile
from concourse import bacc, bass_utils

B, T, H, S = 64, 512, 768, 128
N_CORES = 8
BL = B // N_CORES  # local batches per core
P = 128
KT = T // P        # t-tiles per batch
KH = H // P        # h-tiles
R = BL * S         # MLP rows per core
RC = 2 * S         # moving-dim chunk (2 batches)
NRC = R // RC

_CACHE: dict = {}


def _build_program():
    f32, f16 = mybir.dt.float32, mybir.dt.float16
    FT = mybir.ActivationFunctionType
    OP = mybir.AluOpType

    nc = bacc.Bacc("TRN2", target_bir_lowering=False, debug=False)
    hid = nc.dram_tensor("hidden", [BL, P, KT * H], f16, kind="ExternalInput").ap()
    mtp = nc.dram_tensor("mtp", [P, BL * KT * S], f16, kind="ExternalInput").ap()
    w1 = nc.dram_tensor("w1", [P, KH * H], f16, kind="ExternalInput").ap()
    w2 = nc.dram_tensor("w2", [P, KH * H], f16, kind="ExternalInput").ap()
    wpack = nc.dram_tensor("wpack", [P, KH], f16, kind="ExternalInput").ap()
    cpack = nc.dram_tensor("cpack", [P, 13], f32, kind="ExternalInput").ap()
    out = nc.dram_tensor("out", [BL, S], f32, kind="ExternalOutput").ap()

    with tile.TileContext(nc) as tc:
        with (
            tc.tile_pool(name="consts", bufs=1) as consts,
            tc.tile_pool(name="wpool", bufs=1) as wpool,
            tc.tile_pool(name="hpool", bufs=1) as hpool,
            tc.tile_pool(name="xtpool", bufs=1) as xtpool,
            tc.tile_pool(name="ypool", bufs=1) as ypool,
            tc.tile_pool(name="ps", bufs=8, space="PSUM") as ps,
        ):
            # Two HWDGE queues (sync + ACT) transfer in parallel; each queue
            # is serial in issue order, so the streams are interleaved to
            # match compute-consumption order on both queues.
            cpf_sb = consts.tile([P, 13], f32)
            nc.scalar.dma_start(cpf_sb, cpack)
            w3_sb = consts.tile([P, KH], f16, name="w3_sb")
            nc.scalar.dma_start(w3_sb, wpack)
            b1_sb = cpf_sb[:, 0:KH]
            b2_sb = cpf_sb[:, KH : 2 * KH]
            b3_sb = cpf_sb[0:1, 12:13]

            mt_sb = hpool.tile([P, BL * KT * S], f16, name="mt_sb")
            hbs = [None] * BL

            def load_mt(eng, b0, b1):
                eng.dma_start(
                    mt_sb[:, b0 * KT * S : b1 * KT * S],
                    mtp[:, b0 * KT * S : b1 * KT * S],
                )

            def load_hb(eng, b):
                if b < 2:
                    # first batches arrive per t-tile so pooling starts on the
                    # first 0.2 MB instead of the full 0.8 MB batch
                    tiles = []
                    for k in range(KT):
                        t = hpool.tile([P, H], f16, tag=f"hb{b}k{k}", name=f"hb{b}k{k}")
                        eng.dma_start(t, hid[b][:, k * H : (k + 1) * H])
                        tiles.append(t)
                    hbs[b] = tiles
                else:
                    hb = hpool.tile([P, KT * H], f16, tag=f"hb{b}", name=f"hb{b}")
                    eng.dma_start(hb, hid[b])
                    hbs[b] = hb

            def hb_tile(b, k, m):
                if b < 2:
                    return hbs[b][k][:, m * P : (m + 1) * P]
                return hbs[b][:, k * H + m * P : k * H + (m + 1) * P]

            def mt_tile(b, k):
                return mt_sb[:, (b * KT + k) * S : (b * KT + k + 1) * S]

            # sync queue: the batches the pooling front consumes first
            load_mt(nc.sync, 0, 1)
            load_hb(nc.sync, 0)
            load_mt(nc.sync, 1, 2)
            load_hb(nc.sync, 1)
            load_mt(nc.sync, 2, BL)
            load_hb(nc.sync, 2)
            load_hb(nc.sync, 3)
            load_hb(nc.sync, 4)
            # ACT queue: weights, then the tail batches
            w1_sb = wpool.tile([P, KH * H], f16, name="w1_sb")
            nc.scalar.dma_start(w1_sb, w1)
            w2_sb = wpool.tile([P, KH * H], f16, name="w2_sb")
            nc.scalar.dma_start(w2_sb, w2)
            load_hb(nc.scalar, 5)
            load_hb(nc.scalar, 6)
            load_hb(nc.scalar, 7)

            xts = [xtpool.tile([P, R], f16, tag=f"xt{k}", name=f"xt{k}") for k in range(KH)]
            y1s = [ypool.tile([P, R], f16, tag=f"y1_{m}", name=f"y1_{m}") for m in range(KH)]
            y2s = [ypool.tile([P, R], f16, tag=f"y2_{m}", name=f"y2_{m}") for m in range(KH)]
            pred_t = ypool.tile([1, R], f32, tag="pred_t")
            pred = ypool.tile([1, R], f32, tag="pred")

            def pool_b(b):
                psums = [ps.tile([P, S], f32, tag="ps", name=f"pp{b}_{m}") for m in range(KH)]
                if b < 2:
                    # k-outer: fire all h-tiles of an arrived t-chunk at once
                    for k in range(KT):
                        for m in range(KH):
                            nc.tensor.matmul(
                                psums[m],
                                lhsT=hb_tile(b, k, m),
                                rhs=mt_tile(b, k),
                                start=(k == 0),
                                stop=(k == KT - 1),
                            )
                else:
                    # m-outer: short psum lifetimes
                    for m in range(KH):
                        for k in range(KT):
                            nc.tensor.matmul(
                                psums[m],
                                lhsT=hb_tile(b, k, m),
                                rhs=mt_tile(b, k),
                                start=(k == 0),
                                stop=(k == KT - 1),
                            )
                for m in range(KH):
                    nc.vector.tensor_copy(xts[m][:, b * S : (b + 1) * S], psums[m])

            def fc(w_sb, b_sb, xs, outs, rc, func):
                for m in range(KH):
                    pt = ps.tile([P, RC], f32, tag="ps", name=f"fc{rc}_{m}")
                    for k in range(KH):
                        nc.tensor.matmul(
                            pt,
                            lhsT=w_sb[:, k * H + m * P : k * H + (m + 1) * P],
                            rhs=xs[k][:, rc * RC : (rc + 1) * RC],
                            start=(k == 0),
                            stop=(k == KH - 1),
                        )
                    nc.scalar.activation(
                        outs[m][:, rc * RC : (rc + 1) * RC],
                        pt,
                        func,
                        bias=b_sb[:, m : m + 1],
                    )

            def fc3(rc):
                pt = ps.tile([1, RC], f32, tag="ps", name=f"fc3_{rc}")
                for k in range(KH):
                    nc.tensor.matmul(
                        pt,
                        lhsT=w3_sb[:, k : k + 1],
                        rhs=y2s[k][:, rc * RC : (rc + 1) * RC],
                        start=(k == 0),
                        stop=(k == KH - 1),
                    )
                # sigmoid(z) = 0.5 + 0.5*tanh(z/2); w3/b3 pre-scaled by 0.5
                nc.scalar.activation(
                    pred_t[:, rc * RC : (rc + 1) * RC],
                    pt,
                    FT.Tanh,
                    bias=b3_sb,
                )
                nc.vector.tensor_scalar(
                    pred[:, rc * RC : (rc + 1) * RC],
                    pred_t[:, rc * RC : (rc + 1) * RC],
                    0.5,
                    0.5,
                    OP.mult,
                    OP.add,
                )
                # stream this chunk's predictions out immediately; only the
                # final 1 KB remains on the critical path after the last tanh
                nc.sync.dma_start(
                    out.rearrange("b s -> (b s)")[rc * RC : (rc + 1) * RC],
                    pred[:, rc * RC : (rc + 1) * RC],
                )

            gelu = FT.Gelu
            pool_b(0)
            pool_b(1)
            fc(w1_sb, b1_sb, xts, y1s, 0, gelu)
            pool_b(2)
            pool_b(3)
            fc(w1_sb, b1_sb, xts, y1s, 1, gelu)
            fc(w2_sb, b2_sb, y1s, y2s, 0, gelu)
            fc3(0)
            pool_b(4)
            pool_b(5)
            fc(w1_sb, b1_sb, xts, y1s, 2, gelu)
            fc(w2_sb, b2_sb, y1s, y2s, 1, gelu)
            fc3(1)
            pool_b(6)
            pool_b(7)
            fc(w1_sb, b1_sb, xts, y1s, 3, gelu)
            fc(w2_sb, b2_sb, y1s, y2s, 2, gelu)
            fc3(2)
            fc(w2_sb, b2_sb, y1s, y2s, 3, gelu)
            fc3(3)

    nc.compile()
    return nc


def _get_program():
    if "nc" not in _CACHE:
        _CACHE["nc"] = _build_program()
    return _CACHE["nc"]


def make_in_maps(hidden, statements_ids, w1, b1, w2, b2, w3, b3):
    hidden = np.asarray(hidden, dtype=np.float32)
    sid = np.asarray(statements_ids, dtype=np.int32)

    # [B, P, KT*H] fp16, partition = token-within-tile
    hid16 = (
        hidden.astype(np.float16)
        .reshape(B, KT, P, H)
        .transpose(0, 2, 1, 3)
        .reshape(B, P, KT * H)
    )
    # count-normalized one-hot: mt[b, t, s] = (sid[b,t]==s) / max(cnt[b,s], 1)
    onehot = sid[:, :, None] == np.arange(S, dtype=np.int32)[None, None, :]
    cnt = onehot.sum(axis=1).astype(np.float32)
    inv = 1.0 / np.maximum(cnt, 1.0)
    mtp = (
        (onehot.astype(np.float32) * inv[:, None, :])
        .astype(np.float16)
        .reshape(B, KT, P, S)
        .transpose(0, 2, 1, 3)  # [B, P, KT, S]
        .reshape(B, P, KT * S)
    )

    def packw(w):
        return np.ascontiguousarray(
            np.asarray(w, np.float32)
            .astype(np.float16)
            .reshape(KH, P, H)
            .transpose(1, 0, 2)
            .reshape(P, KH * H)
        )

    w1p, w2p = packw(w1), packw(w2)
    # sigmoid-as-tanh: pre-scale the last layer by 0.5
    wpack = np.ascontiguousarray(
        (0.5 * np.asarray(w3, np.float32)).astype(np.float16).reshape(KH, P).T
    )
    cpack = np.zeros((P, 13), dtype=np.float32)
    cpack[:, 0:KH] = np.asarray(b1, np.float32).reshape(KH, P).T
    cpack[:, KH : 2 * KH] = np.asarray(b2, np.float32).reshape(KH, P).T
    cpack[0, 12] = np.float32(0.5 * np.asarray(b3).reshape(-1)[0])

    in_maps = []
    for c in range(N_CORES):
        mtp_c = (
            mtp[c * BL : (c + 1) * BL]
            .transpose(1, 0, 2)  # [P, BL, KT*S]
            .reshape(P, BL * KT * S)
        )
        in_maps.append(
            {
                "hidden": np.ascontiguousarray(hid16[c * BL : (c + 1) * BL]),
                "mtp": np.ascontiguousarray(mtp_c),
                "w1": w1p,
                "w2": w2p,
                "wpack": wpack,
                "cpack": cpack,
            }
        )
    return in_maps


def kernel(hidden, statements_ids, w1, b1, w2, b2, w3, b3, **kwargs):
    nc = _get_program()
    in_maps = make_in_maps(hidden, statements_ids, w1, b1, w2, b2, w3, b3)
    trace = bool(int(os.environ.get("KERNEL_TRACE", "0")))
    res = bass_utils.run_bass_kernel_spmd(
        nc, in_maps, core_ids=list(range(N_CORES)), trace=trace
    )
    _CACHE["last_results"] = res
    out = np.concatenate([res.results[c]["out"] for c in range(N_CORES)], axis=0)
    return out.astype(np.float32)


# revision 7
# speedup vs baseline: 1.0319x; 1.0319x over previous
"""Trainium2 Bass kernel: per-batch segment-mean pooling + 3-layer MLP.

Reference computation (B=64, T=512, H=768, S=128):
  pooled[b,s,:] = mean over t of hidden[b,t,:] where statements_ids[b,t]==s
  x = gelu(pooled @ w1 + b1); x = gelu(x @ w2 + b2)
  out[b,s] = sigmoid(x @ w3 + b3)

Distribution: data-parallel over batch across 8 NeuronCores (8 batches per
core); MLP weights replicated.

Per-core algorithm:
  - Host precomputes the count-normalized one-hot MT[t,s] = inv[s]*(sid[t]==s)
    (counts depend only on statements_ids) and ships it in fp16, along with
    fp16 hidden and weights.  fp16 keeps the PE at 1 cycle/row for any moving
    size and halves HBM traffic vs fp32; 10 mantissa bits keep the rel err
    ~1e-3 (tolerance 2e-2).
  - pooled^T tiles directly: matmul(lhsT=hidden[t,h-tile], rhs=MT[t,s])
    -> psum [128h, 128s], accumulated over the 4 t-tiles.  No PE transposes,
    no on-device one-hot build, no normalization chain: the psum already
    holds mean-pooled values in the [h, s] orientation the MLP wants.
  - psum -> SBUF copies (fp16) on DVE (GPSIMD can't read PSUM on TRN2).
  - MLP batched over all 8 local batches: rows = 8*128 = 1024 moving dim,
    weights stationary; gelu + bias fused on ACT.
  - sigmoid(z) = 0.5 + 0.5*tanh(z/2): tanh lives in the same ACT function
    table as gelu, so the 8 x 1.28us ACT_TABLE_LOAD thrash of alternating
    gelu/sigmoid tables disappears.  w3/b3 are pre-scaled by 0.5 on the
    host; the final 0.5*t+0.5 affine runs on the otherwise-idle DVE.
  - DMA issue (~600ns per descriptor on an engine's HWDGE queue) is split:
    weights on the ACT queue, the hidden/MT stream on sync, so neither
    stream's issue latency delays the other's arrival.
"""

import os
import sys

sys.path.insert(0, "/opt/trn_rl_repo")

import numpy as np

import concourse.bass as bass
import concourse.mybir as mybir
import concourse.tile as tile
from concourse import bacc, bass_utils

B, T, H, S = 64, 512, 768, 128
N_CORES = 8
BL = B // N_CORES  # local batches per core
P = 128
KT = T // P        # t-tiles per batch
KH = H // P        # h-tiles
R = BL * S         # MLP rows per core
RC = 2 * S         # moving-dim chunk (2 batches)
NRC = R // RC

_CACHE: dict = {}


def _build_program():
    f32, f16 = mybir.dt.float32, mybir.dt.float16
    FT = mybir.ActivationFunctionType
    OP = mybir.AluOpType

    nc = bacc.Bacc("TRN2", target_bir_lowering=False, debug=False)
    hid = nc.dram_tensor("hidden", [BL, P, KT * H], f16, kind="ExternalInput").ap()
    mtp = nc.dram_tensor("mtp", [P, BL * KT * S], f16, kind="ExternalInput").ap()
    w1 = nc.dram_tensor("w1", [P, KH * H], f16, kind="ExternalInput").ap()
    w2 = nc.dram_tensor("w2", [P, KH * H], f16, kind="ExternalInput").ap()
    wpack = nc.dram_tensor("wpack", [P, KH], f16, kind="ExternalInput").ap()
    cpack = nc.dram_tensor("cpack", [P, 13], f32, kind="ExternalInput").ap()
    out = nc.dram_tensor("out", [BL, S], f32, kind="ExternalOutput").ap()

    with tile.TileContext(nc) as tc:
        with (
            tc.tile_pool(name="consts", bufs=1) as consts,
            tc.tile_pool(name="wpool", bufs=1) as wpool,
            tc.tile_pool(name="hpool", bufs=1) as hpool,
            tc.tile_pool(name="xtpool", bufs=1) as xtpool,
            tc.tile_pool(name="ypool", bufs=1) as ypool,
            tc.tile_pool(name="ps", bufs=8, space="PSUM") as ps,
        ):
            # Two HWDGE queues (sync + ACT) transfer in parallel; each queue
            # is serial in issue order, so the streams are interleaved to
            # match compute-consumption order on both queues.
            cpf_sb = consts.tile([P, 13], f32)
            nc.scalar.dma_start(cpf_sb, cpack)
            w3_sb = consts.tile([P, KH], f16, name="w3_sb")
            nc.scalar.dma_start(w3_sb, wpack)
            b1_sb = cpf_sb[:, 0:KH]
            b2_sb = cpf_sb[:, KH : 2 * KH]
            b3_sb = cpf_sb[0:1, 12:13]

            mt_sb = hpool.tile([P, BL * KT * S], f16, name="mt_sb")
            hbs = [None] * BL

            def load_mt(eng, b0, b1):
                eng.dma_start(
                    mt_sb[:, b0 * KT * S : b1 * KT * S],
                    mtp[:, b0 * KT * S : b1 * KT * S],
                )

            def load_hb(eng, b):
                if b < 2:
                    # first batches arrive per t-tile so pooling starts on the
                    # first 0.2 MB instead of the full 0.8 MB batch
                    tiles = []
                    for k in range(KT):
                        t = hpool.tile([P, H], f16, tag=f"hb{b}k{k}", name=f"hb{b}k{k}")
                        eng.dma_start(t, hid[b][:, k * H : (k + 1) * H])
                        tiles.append(t)
                    hbs[b] = tiles
                else:
                    hb = hpool.tile([P, KT * H], f16, tag=f"hb{b}", name=f"hb{b}")
                    eng.dma_start(hb, hid[b])
                    hbs[b] = hb

            def hb_tile(b, k, m):
                if b < 2:
                    return hbs[b][k][:, m * P : (m + 1) * P]
                return hbs[b][:, k * H + m * P : k * H + (m + 1) * P]

            def mt_tile(b, k):
                return mt_sb[:, (b * KT + k) * S : (b * KT + k + 1) * S]

            # Three DMA paths share the 16 DMA engines but each logical
            # queue caps out well below the HBM feed, so the 9.7MB input is
            # balanced ~evenly across sync HWDGE, ACT HWDGE, and the
            # otherwise-idle GPSIMD SWDGE.
            # sync: the batches the pooling front consumes first
            load_mt(nc.sync, 0, 1)
            load_hb(nc.sync, 0)
            load_mt(nc.sync, 1, 2)
            load_hb(nc.sync, 1)
            load_mt(nc.sync, 2, BL)
            load_hb(nc.sync, 5)
            load_hb(nc.sync, 6)
            # ACT queue: weights first (fc1 needs w1 by ~15us), one tail batch
            w1_sb = wpool.tile([P, KH * H], f16, name="w1_sb")
            nc.scalar.dma_start(w1_sb, w1)
            w2_sb = wpool.tile([P, KH * H], f16, name="w2_sb")
            nc.scalar.dma_start(w2_sb, w2)
            load_hb(nc.scalar, 7)
            # GPSIMD software DGE: the mid batches
            load_hb(nc.gpsimd, 2)
            load_hb(nc.gpsimd, 3)
            load_hb(nc.gpsimd, 4)

            xts = [xtpool.tile([P, R], f16, tag=f"xt{k}", name=f"xt{k}") for k in range(KH)]
            y1s = [ypool.tile([P, R], f16, tag=f"y1_{m}", name=f"y1_{m}") for m in range(KH)]
            y2s = [ypool.tile([P, R], f16, tag=f"y2_{m}", name=f"y2_{m}") for m in range(KH)]
            pred_t = ypool.tile([1, R], f32, tag="pred_t")
            pred = ypool.tile([1, R], f32, tag="pred")

            def pool_b(b):
                psums = [ps.tile([P, S], f32, tag="ps", name=f"pp{b}_{m}") for m in range(KH)]
                if b < 2:
                    # k-outer: fire all h-tiles of an arrived t-chunk at once
                    for k in range(KT):
                        for m in range(KH):
                            nc.tensor.matmul(
                                psums[m],
                                lhsT=hb_tile(b, k, m),
                                rhs=mt_tile(b, k),
                                start=(k == 0),
                                stop=(k == KT - 1),
                            )
                else:
                    # m-outer: short psum lifetimes
                    for m in range(KH):
                        for k in range(KT):
                            nc.tensor.matmul(
                                psums[m],
                                lhsT=hb_tile(b, k, m),
                                rhs=mt_tile(b, k),
                                start=(k == 0),
                                stop=(k == KT - 1),
                            )
                for m in range(KH):
                    nc.vector.tensor_copy(xts[m][:, b * S : (b + 1) * S], psums[m])

            def fc(w_sb, b_sb, xs, outs, rc, func):
                for m in range(KH):
                    pt = ps.tile([P, RC], f32, tag="ps", name=f"fc{rc}_{m}")
                    for k in range(KH):
                        nc.tensor.matmul(
                            pt,
                            lhsT=w_sb[:, k * H + m * P : k * H + (m + 1) * P],
                            rhs=xs[k][:, rc * RC : (rc + 1) * RC],
                            start=(k == 0),
                            stop=(k == KH - 1),
                        )
                    nc.scalar.activation(
                        outs[m][:, rc * RC : (rc + 1) * RC],
                        pt,
                        func,
                        bias=b_sb[:, m : m + 1],
                    )

            def fc3(rc):
                pt = ps.tile([1, RC], f32, tag="ps", name=f"fc3_{rc}")
                for k in range(KH):
                    nc.tensor.matmul(
                        pt,
                        lhsT=w3_sb[:, k : k + 1],
                        rhs=y2s[k][:, rc * RC : (rc + 1) * RC],
                        start=(k == 0),
                        stop=(k == KH - 1),
                    )
                # sigmoid(z) = 0.5 + 0.5*tanh(z/2); w3/b3 pre-scaled by 0.5
                nc.scalar.activation(
                    pred_t[:, rc * RC : (rc + 1) * RC],
                    pt,
                    FT.Tanh,
                    bias=b3_sb,
                )
                nc.vector.tensor_scalar(
                    pred[:, rc * RC : (rc + 1) * RC],
                    pred_t[:, rc * RC : (rc + 1) * RC],
                    0.5,
                    0.5,
                    OP.mult,
                    OP.add,
                )
                # stream this chunk's predictions out immediately; only the
                # final 1 KB remains on the critical path after the last tanh
                nc.sync.dma_start(
                    out.rearrange("b s -> (b s)")[rc * RC : (rc + 1) * RC],
                    pred[:, rc * RC : (rc + 1) * RC],
                )

            gelu = FT.Gelu
            pool_b(0)
            pool_b(1)
            fc(w1_sb, b1_sb, xts, y1s, 0, gelu)
            pool_b(2)
            pool_b(3)
            fc(w1_sb, b1_sb, xts, y1s, 1, gelu)
            fc(w2_sb, b2_sb, y1s, y2s, 0, gelu)
            fc3(0)
            pool_b(4)
            pool_b(5)
            fc(w1_sb, b1_sb, xts, y1s, 2, gelu)
            fc(w2_sb, b2_sb, y1s, y2s, 1, gelu)
            fc3(1)
            pool_b(6)
            pool_b(7)
            fc(w1_sb, b1_sb, xts, y1s, 3, gelu)
            fc(w2_sb, b2_sb, y1s, y2s, 2, gelu)
            fc3(2)
            fc(w2_sb, b2_sb, y1s, y2s, 3, gelu)
            fc3(3)

    nc.compile()
    return nc


def _get_program():
    if "nc" not in _CACHE:
        _CACHE["nc"] = _build_program()
    return _CACHE["nc"]


def make_in_maps(hidden, statements_ids, w1, b1, w2, b2, w3, b3):
    hidden = np.asarray(hidden, dtype=np.float32)
    sid = np.asarray(statements_ids, dtype=np.int32)

    # [B, P, KT*H] fp16, partition = token-within-tile
    hid16 = (
        hidden.astype(np.float16)
        .reshape(B, KT, P, H)
        .transpose(0, 2, 1, 3)
        .reshape(B, P, KT * H)
    )
    # count-normalized one-hot: mt[b, t, s] = (sid[b,t]==s) / max(cnt[b,s], 1)
    onehot = sid[:, :, None] == np.arange(S, dtype=np.int32)[None, None, :]
    cnt = onehot.sum(axis=1).astype(np.float32)
    inv = 1.0 / np.maximum(cnt, 1.0)
    mtp = (
        (onehot.astype(np.float32) * inv[:, None, :])
        .astype(np.float16)
        .reshape(B, KT, P, S)
        .transpose(0, 2, 1, 3)  # [B, P, KT, S]
        .reshape(B, P, KT * S)
    )

    def packw(w):
        return np.ascontiguousarray(
            np.asarray(w, np.float32)
            .astype(np.float16)
            .reshape(KH, P, H)
            .transpose(1, 0, 2)
            .reshape(P, KH * H)
        )

    w1p, w2p = packw(w1), packw(w2)
    # sigmoid-as-tanh: pre-scale the last layer by 0.5
    wpack = np.ascontiguousarray(
        (0.5 * np.asarray(w3, np.float32)).astype(np.float16).reshape(KH, P).T
    )
    cpack = np.zeros((P, 13), dtype=np.float32)
    cpack[:, 0:KH] = np.asarray(b1, np.float32).reshape(KH, P).T
    cpack[:, KH : 2 * KH] = np.asarray(b2, np.float32).reshape(KH, P).T
    cpack[0, 12] = np.float32(0.5 * np.asarray(b3).reshape(-1)[0])

    in_maps = []
    for c in range(N_CORES):
        mtp_c = (
            mtp[c * BL : (c + 1) * BL]
            .transpose(1, 0, 2)  # [P, BL, KT*S]
            .reshape(P, BL * KT * S)
        )
        in_maps.append(
            {
                "hidden": np.ascontiguousarray(hid16[c * BL : (c + 1) * BL]),
                "mtp": np.ascontiguousarray(mtp_c),
                "w1": w1p,
                "w2": w2p,
                "wpack": wpack,
                "cpack": cpack,
            }
        )
    return in_maps


def kernel(hidden, statements_ids, w1, b1, w2, b2, w3, b3, **kwargs):
    nc = _get_program()
    in_maps = make_in_maps(hidden, statements_ids, w1, b1, w2, b2, w3, b3)
    trace = bool(int(os.environ.get("KERNEL_TRACE", "0")))
    res = bass_utils.run_bass_kernel_spmd(
        nc, in_maps, core_ids=list(range(N_CORES)), trace=trace
    )
    _CACHE["last_results"] = res
    out = np.concatenate([res.results[c]["out"] for c in range(N_CORES)], axis=0)
    return out.astype(np.float32)


# revision 8
# speedup vs baseline: 1.0569x; 1.0242x over previous
"""Trainium2 Bass kernel: per-batch segment-mean pooling + 3-layer MLP.

Reference computation (B=64, T=512, H=768, S=128):
  pooled[b,s,:] = mean over t of hidden[b,t,:] where statements_ids[b,t]==s
  x = gelu(pooled @ w1 + b1); x = gelu(x @ w2 + b2)
  out[b,s] = sigmoid(x @ w3 + b3)

Distribution: data-parallel over batch across 8 NeuronCores (8 batches per
core); MLP weights replicated.

Per-core algorithm:
  - Host precomputes the count-normalized one-hot MT[t,s] = inv[s]*(sid[t]==s)
    (counts depend only on statements_ids) and ships it in fp16, along with
    fp16 hidden and weights.  fp16 keeps the PE at 1 cycle/row for any moving
    size and halves HBM traffic vs fp32; 10 mantissa bits keep the rel err
    ~1e-3 (tolerance 2e-2).
  - pooled^T tiles directly: matmul(lhsT=hidden[t,h-tile], rhs=MT[t,s])
    -> psum [128h, 128s], accumulated over the 4 t-tiles.  No PE transposes,
    no on-device one-hot build, no normalization chain: the psum already
    holds mean-pooled values in the [h, s] orientation the MLP wants.
  - psum -> SBUF copies (fp16) on DVE (GPSIMD can't read PSUM on TRN2).
  - MLP batched over all 8 local batches: rows = 8*128 = 1024 moving dim,
    weights stationary; gelu + bias fused on ACT.
  - sigmoid(z) = 0.5 + 0.5*tanh(z/2): tanh lives in the same ACT function
    table as gelu, so the 8 x 1.28us ACT_TABLE_LOAD thrash of alternating
    gelu/sigmoid tables disappears.  w3/b3 are pre-scaled by 0.5 on the
    host; the final 0.5*t+0.5 affine runs on the otherwise-idle DVE.
  - DMA issue (~600ns per descriptor on an engine's HWDGE queue) is split:
    weights on the ACT queue, the hidden/MT stream on sync, so neither
    stream's issue latency delays the other's arrival.
"""

import os
import sys

sys.path.insert(0, "/opt/trn_rl_repo")

import numpy as np

import concourse.bass as bass
import concourse.mybir as mybir
import concourse.tile as tile
from concourse import bacc, bass_utils

B, T, H, S = 64, 512, 768, 128
N_CORES = 8
BL = B // N_CORES  # local batches per core
P = 128
KT = T // P        # t-tiles per batch
KH = H // P        # h-tiles
R = BL * S         # MLP rows per core
RC = 2 * S         # moving-dim chunk (2 batches)
NRC = R // RC

_CACHE: dict = {}


def _build_program():
    f32, f16 = mybir.dt.float32, mybir.dt.float16
    FT = mybir.ActivationFunctionType
    OP = mybir.AluOpType

    nc = bacc.Bacc("TRN2", target_bir_lowering=False, debug=False)
    hid = nc.dram_tensor("hidden", [BL, P, KT * H], f16, kind="ExternalInput").ap()
    mtp = nc.dram_tensor("mtp", [P, BL * KT * S], f16, kind="ExternalInput").ap()
    w1 = nc.dram_tensor("w1", [P, KH * H], f16, kind="ExternalInput").ap()
    w2 = nc.dram_tensor("w2", [P, KH * H], f16, kind="ExternalInput").ap()
    wpack = nc.dram_tensor("wpack", [P, KH], f16, kind="ExternalInput").ap()
    cpack = nc.dram_tensor("cpack", [P, 13], f32, kind="ExternalInput").ap()
    out = nc.dram_tensor("out", [BL, S], f32, kind="ExternalOutput").ap()

    with tile.TileContext(nc) as tc:
        with (
            tc.tile_pool(name="consts", bufs=1) as consts,
            tc.tile_pool(name="wpool", bufs=1) as wpool,
            tc.tile_pool(name="hpool", bufs=1) as hpool,
            tc.tile_pool(name="xtpool", bufs=1) as xtpool,
            tc.tile_pool(name="ypool", bufs=1) as ypool,
            tc.tile_pool(name="ps", bufs=8, space="PSUM") as ps,
        ):
            # Two HWDGE queues (sync + ACT) transfer in parallel; each queue
            # is serial in issue order, so the streams are interleaved to
            # match compute-consumption order on both queues.
            cpf_sb = consts.tile([P, 13], f32)
            nc.scalar.dma_start(cpf_sb, cpack)
            w3_sb = consts.tile([P, KH], f16, name="w3_sb")
            nc.scalar.dma_start(w3_sb, wpack)
            b1_sb = cpf_sb[:, 0:KH]
            b2_sb = cpf_sb[:, KH : 2 * KH]
            b3_sb = cpf_sb[0:1, 12:13]

            mt_sb = hpool.tile([P, BL * KT * S], f16, name="mt_sb")
            hbs = [None] * BL

            def load_mt(eng, b0, b1):
                eng.dma_start(
                    mt_sb[:, b0 * KT * S : b1 * KT * S],
                    mtp[:, b0 * KT * S : b1 * KT * S],
                )

            def load_hb(eng, b):
                if b < 2:
                    # first batches arrive per t-tile so pooling starts on the
                    # first 0.2 MB instead of the full 0.8 MB batch
                    tiles = []
                    for k in range(KT):
                        t = hpool.tile([P, H], f16, tag=f"hb{b}k{k}", name=f"hb{b}k{k}")
                        eng.dma_start(t, hid[b][:, k * H : (k + 1) * H])
                        tiles.append(t)
                    hbs[b] = tiles
                else:
                    hb = hpool.tile([P, KT * H], f16, tag=f"hb{b}", name=f"hb{b}")
                    eng.dma_start(hb, hid[b])
                    hbs[b] = hb

            def hb_tile(b, k, m):
                if b < 2:
                    return hbs[b][k][:, m * P : (m + 1) * P]
                return hbs[b][:, k * H + m * P : k * H + (m + 1) * P]

            def mt_tile(b, k):
                return mt_sb[:, (b * KT + k) * S : (b * KT + k + 1) * S]

            # sync: the batches the pooling front consumes first; hid1 rides
            # the otherwise-idle GPSIMD queue in parallel with hid0 so
            # pool(1) doesn't wait behind hid0's transfers.
            load_mt(nc.sync, 0, 1)
            load_hb(nc.sync, 0)
            load_mt(nc.sync, 1, 2)
            load_hb(nc.gpsimd, 1)
            load_mt(nc.sync, 2, BL)
            for b in range(2, BL - 1):
                load_hb(nc.sync, b)
            # ACT queue: weights first (fc1 needs w1 by ~15us), one tail batch
            w1_sb = wpool.tile([P, KH * H], f16, name="w1_sb")
            nc.scalar.dma_start(w1_sb, w1)
            w2_sb = wpool.tile([P, KH * H], f16, name="w2_sb")
            nc.scalar.dma_start(w2_sb, w2)
            load_hb(nc.scalar, BL - 1)

            xts = [xtpool.tile([P, R], f16, tag=f"xt{k}", name=f"xt{k}") for k in range(KH)]
            y1s = [ypool.tile([P, R], f16, tag=f"y1_{m}", name=f"y1_{m}") for m in range(KH)]
            y2s = [ypool.tile([P, R], f16, tag=f"y2_{m}", name=f"y2_{m}") for m in range(KH)]
            pred_t = ypool.tile([1, R], f32, tag="pred_t")
            pred = ypool.tile([1, R], f32, tag="pred")

            def pool_b(b):
                psums = [ps.tile([P, S], f32, tag="ps", name=f"pp{b}_{m}") for m in range(KH)]
                if b < 2:
                    # k-outer: fire all h-tiles of an arrived t-chunk at once
                    for k in range(KT):
                        for m in range(KH):
                            nc.tensor.matmul(
                                psums[m],
                                lhsT=hb_tile(b, k, m),
                                rhs=mt_tile(b, k),
                                start=(k == 0),
                                stop=(k == KT - 1),
                            )
                else:
                    # m-outer: short psum lifetimes
                    for m in range(KH):
                        for k in range(KT):
                            nc.tensor.matmul(
                                psums[m],
                                lhsT=hb_tile(b, k, m),
                                rhs=mt_tile(b, k),
                                start=(k == 0),
                                stop=(k == KT - 1),
                            )
                for m in range(KH):
                    nc.vector.tensor_copy(xts[m][:, b * S : (b + 1) * S], psums[m])

            def fc(w_sb, b_sb, xs, outs, rc, func):
                for m in range(KH):
                    pt = ps.tile([P, RC], f32, tag="ps", name=f"fc{rc}_{m}")
                    for k in range(KH):
                        nc.tensor.matmul(
                            pt,
                            lhsT=w_sb[:, k * H + m * P : k * H + (m + 1) * P],
                            rhs=xs[k][:, rc * RC : (rc + 1) * RC],
                            start=(k == 0),
                            stop=(k == KH - 1),
                        )
                    nc.scalar.activation(
                        outs[m][:, rc * RC : (rc + 1) * RC],
                        pt,
                        func,
                        bias=b_sb[:, m : m + 1],
                    )

            def fc3(rc):
                pt = ps.tile([1, RC], f32, tag="ps", name=f"fc3_{rc}")
                for k in range(KH):
                    nc.tensor.matmul(
                        pt,
                        lhsT=w3_sb[:, k : k + 1],
                        rhs=y2s[k][:, rc * RC : (rc + 1) * RC],
                        start=(k == 0),
                        stop=(k == KH - 1),
                    )
                # sigmoid(z) = 0.5 + 0.5*tanh(z/2); w3/b3 pre-scaled by 0.5
                nc.scalar.activation(
                    pred_t[:, rc * RC : (rc + 1) * RC],
                    pt,
                    FT.Tanh,
                    bias=b3_sb,
                )
                nc.vector.tensor_scalar(
                    pred[:, rc * RC : (rc + 1) * RC],
                    pred_t[:, rc * RC : (rc + 1) * RC],
                    0.5,
                    0.5,
                    OP.mult,
                    OP.add,
                )
                # stream this chunk's predictions out immediately; only the
                # final 1 KB remains on the critical path after the last tanh
                nc.sync.dma_start(
                    out.rearrange("b s -> (b s)")[rc * RC : (rc + 1) * RC],
                    pred[:, rc * RC : (rc + 1) * RC],
                )

            gelu = FT.Gelu
            pool_b(0)
            pool_b(1)
            fc(w1_sb, b1_sb, xts, y1s, 0, gelu)
            pool_b(2)
            pool_b(3)
            fc(w1_sb, b1_sb, xts, y1s, 1, gelu)
            fc(w2_sb, b2_sb, y1s, y2s, 0, gelu)
            fc3(0)
            pool_b(4)
            pool_b(5)
            fc(w1_sb, b1_sb, xts, y1s, 2, gelu)
            fc(w2_sb, b2_sb, y1s, y2s, 1, gelu)
            fc3(1)
            pool_b(6)
            pool_b(7)
            fc(w1_sb, b1_sb, xts, y1s, 3, gelu)
            fc(w2_sb, b2_sb, y1s, y2s, 2, gelu)
            fc3(2)
            fc(w2_sb, b2_sb, y1s, y2s, 3, gelu)
            fc3(3)

    nc.compile()
    return nc


def _get_program():
    if "nc" not in _CACHE:
        _CACHE["nc"] = _build_program()
    return _CACHE["nc"]


def make_in_maps(hidden, statements_ids, w1, b1, w2, b2, w3, b3):
    hidden = np.asarray(hidden, dtype=np.float32)
    sid = np.asarray(statements_ids, dtype=np.int32)

    # [B, P, KT*H] fp16, partition = token-within-tile
    hid16 = (
        hidden.astype(np.float16)
        .reshape(B, KT, P, H)
        .transpose(0, 2, 1, 3)
        .reshape(B, P, KT * H)
    )
    # count-normalized one-hot: mt[b, t, s] = (sid[b,t]==s) / max(cnt[b,s], 1)
    onehot = sid[:, :, None] == np.arange(S, dtype=np.int32)[None, None, :]
    cnt = onehot.sum(axis=1).astype(np.float32)
    inv = 1.0 / np.maximum(cnt, 1.0)
    mtp = (
        (onehot.astype(np.float32) * inv[:, None, :])
        .astype(np.float16)
        .reshape(B, KT, P, S)
        .transpose(0, 2, 1, 3)  # [B, P, KT, S]
        .reshape(B, P, KT * S)
    )

    def packw(w):
        return np.ascontiguousarray(
            np.asarray(w, np.float32)
            .astype(np.float16)
            .reshape(KH, P, H)
            .transpose(1, 0, 2)
            .reshape(P, KH * H)
        )

    w1p, w2p = packw(w1), packw(w2)
    # sigmoid-as-tanh: pre-scale the last layer by 0.5
    wpack = np.ascontiguousarray(
        (0.5 * np.asarray(w3, np.float32)).astype(np.float16).reshape(KH, P).T
    )
    cpack = np.zeros((P, 13), dtype=np.float32)
    cpack[:, 0:KH] = np.asarray(b1, np.float32).reshape(KH, P).T
    cpack[:, KH : 2 * KH] = np.asarray(b2, np.float32).reshape(KH, P).T
    cpack[0, 12] = np.float32(0.5 * np.asarray(b3).reshape(-1)[0])

    in_maps = []
    for c in range(N_CORES):
        mtp_c = (
            mtp[c * BL : (c + 1) * BL]
            .transpose(1, 0, 2)  # [P, BL, KT*S]
            .reshape(P, BL * KT * S)
        )
        in_maps.append(
            {
                "hidden": np.ascontiguousarray(hid16[c * BL : (c + 1) * BL]),
                "mtp": np.ascontiguousarray(mtp_c),
                "w1": w1p,
                "w2": w2p,
                "wpack": wpack,
                "cpack": cpack,
            }
        )
    return in_maps


def kernel(hidden, statements_ids, w1, b1, w2, b2, w3, b3, **kwargs):
    nc = _get_program()
    in_maps = make_in_maps(hidden, statements_ids, w1, b1, w2, b2, w3, b3)
    trace = bool(int(os.environ.get("KERNEL_TRACE", "0")))
    res = bass_utils.run_bass_kernel_spmd(
        nc, in_maps, core_ids=list(range(N_CORES)), trace=trace
    )
    _CACHE["last_results"] = res
    out = np.concatenate([res.results[c]["out"] for c in range(N_CORES)], axis=0)
    return out.astype(np.float32)


# revision 9
# speedup vs baseline: 1.0707x; 1.0131x over previous
"""Trainium2 Bass kernel: per-batch segment-mean pooling + 3-layer MLP.

Reference computation (B=64, T=512, H=768, S=128):
  pooled[b,s,:] = mean over t of hidden[b,t,:] where statements_ids[b,t]==s
  x = gelu(pooled @ w1 + b1); x = gelu(x @ w2 + b2)
  out[b,s] = sigmoid(x @ w3 + b3)

Distribution: data-parallel over batch across 8 NeuronCores (8 batches per
core); MLP weights replicated.

Per-core algorithm:
  - Host precomputes the count-normalized one-hot MT[t,s] = inv[s]*(sid[t]==s)
    (counts depend only on statements_ids) and ships it in fp16, along with
    fp16 hidden and weights.  fp16 keeps the PE at 1 cycle/row for any moving
    size and halves HBM traffic vs fp32; 10 mantissa bits keep the rel err
    ~1e-3 (tolerance 2e-2).
  - pooled^T tiles directly: matmul(lhsT=hidden[t,h-tile], rhs=MT[t,s])
    -> psum [128h, 128s], accumulated over the 4 t-tiles.  No PE transposes,
    no on-device one-hot build, no normalization chain: the psum already
    holds mean-pooled values in the [h, s] orientation the MLP wants.
  - psum -> SBUF copies (fp16) on DVE (GPSIMD can't read PSUM on TRN2).
  - MLP batched over all 8 local batches: rows = 8*128 = 1024 moving dim,
    weights stationary; gelu + bias fused on ACT.
  - sigmoid(z) = 0.5 + 0.5*tanh(z/2): tanh lives in the same ACT function
    table as gelu, so the 8 x 1.28us ACT_TABLE_LOAD thrash of alternating
    gelu/sigmoid tables disappears.  w3/b3 are pre-scaled by 0.5 on the
    host; the final 0.5*t+0.5 affine runs on the otherwise-idle DVE.
  - DMA issue (~600ns per descriptor on an engine's HWDGE queue) is split:
    weights on the ACT queue, the hidden/MT stream on sync, so neither
    stream's issue latency delays the other's arrival.
"""

import os
import sys

sys.path.insert(0, "/opt/trn_rl_repo")

import numpy as np

import concourse.bass as bass
import concourse.mybir as mybir
import concourse.tile as tile
from concourse import bacc, bass_utils

B, T, H, S = 64, 512, 768, 128
N_CORES = 8
BL = B // N_CORES  # local batches per core
P = 128
KT = T // P        # t-tiles per batch
KH = H // P        # h-tiles
R = BL * S         # MLP rows per core
RC = 2 * S         # moving-dim chunk (2 batches)
NRC = R // RC

_CACHE: dict = {}


def _build_program():
    f32, f16 = mybir.dt.float32, mybir.dt.float16
    FT = mybir.ActivationFunctionType
    OP = mybir.AluOpType

    nc = bacc.Bacc("TRN2", target_bir_lowering=False, debug=False)
    hid = nc.dram_tensor("hidden", [BL, P, KT * H], f16, kind="ExternalInput").ap()
    mtp = nc.dram_tensor("mtp", [P, BL * KT * S], f16, kind="ExternalInput").ap()
    w1 = nc.dram_tensor("w1", [P, KH * H], f16, kind="ExternalInput").ap()
    w2 = nc.dram_tensor("w2", [P, KH * H], f16, kind="ExternalInput").ap()
    wpack = nc.dram_tensor("wpack", [P, KH], f16, kind="ExternalInput").ap()
    cpack = nc.dram_tensor("cpack", [P, 13], f32, kind="ExternalInput").ap()
    out = nc.dram_tensor("out", [BL, S], f32, kind="ExternalOutput").ap()

    with tile.TileContext(nc) as tc:
        with (
            tc.tile_pool(name="consts", bufs=1) as consts,
            tc.tile_pool(name="wpool", bufs=1) as wpool,
            tc.tile_pool(name="hpool", bufs=1) as hpool,
            tc.tile_pool(name="xtpool", bufs=1) as xtpool,
            tc.tile_pool(name="ypool", bufs=1) as ypool,
            tc.tile_pool(name="ps", bufs=8, space="PSUM") as ps,
        ):
            # Two HWDGE queues (sync + ACT) transfer in parallel; each queue
            # is serial in issue order, so the streams are interleaved to
            # match compute-consumption order on both queues.
            cpf_sb = consts.tile([P, 13], f32)
            nc.scalar.dma_start(cpf_sb, cpack)
            w3_sb = consts.tile([P, KH], f16, name="w3_sb")
            nc.scalar.dma_start(w3_sb, wpack)
            b1_sb = cpf_sb[:, 0:KH]
            b2_sb = cpf_sb[:, KH : 2 * KH]
            b3_sb = cpf_sb[0:1, 12:13]

            mt_sb = hpool.tile([P, BL * KT * S], f16, name="mt_sb")
            hbs = [None] * BL

            def load_mt(eng, b0, b1):
                eng.dma_start(
                    mt_sb[:, b0 * KT * S : b1 * KT * S],
                    mtp[:, b0 * KT * S : b1 * KT * S],
                )

            def load_hb(eng, b):
                if b < 2:
                    # first batches arrive per t-tile so pooling starts on the
                    # first 0.2 MB instead of the full 0.8 MB batch
                    tiles = []
                    for k in range(KT):
                        t = hpool.tile([P, H], f16, tag=f"hb{b}k{k}", name=f"hb{b}k{k}")
                        eng.dma_start(t, hid[b][:, k * H : (k + 1) * H])
                        tiles.append(t)
                    hbs[b] = tiles
                else:
                    hb = hpool.tile([P, KT * H], f16, tag=f"hb{b}", name=f"hb{b}")
                    eng.dma_start(hb, hid[b])
                    hbs[b] = hb

            def hb_tile(b, k, m):
                if b < 2:
                    return hbs[b][k][:, m * P : (m + 1) * P]
                return hbs[b][:, k * H + m * P : k * H + (m + 1) * P]

            def mt_tile(b, k):
                return mt_sb[:, (b * KT + k) * S : (b * KT + k + 1) * S]

            # sync: the batches the pooling front consumes first; hid1 rides
            # the otherwise-idle GPSIMD queue in parallel with hid0 so
            # pool(1) doesn't wait behind hid0's transfers.
            load_mt(nc.sync, 0, 1)
            load_hb(nc.sync, 0)
            load_mt(nc.sync, 1, 2)
            load_hb(nc.gpsimd, 1)
            load_mt(nc.sync, 2, BL)
            for b in range(2, BL):
                load_hb(nc.sync, b)
            # ACT queue: weights only — a 5th DMA here could hit semaphore
            # rotation and stall the in-order ACT engine's gelus behind it
            w1_sb = wpool.tile([P, KH * H], f16, name="w1_sb")
            nc.scalar.dma_start(w1_sb, w1)
            w2_sb = wpool.tile([P, KH * H], f16, name="w2_sb")
            nc.scalar.dma_start(w2_sb, w2)

            xts = [xtpool.tile([P, R], f16, tag=f"xt{k}", name=f"xt{k}") for k in range(KH)]
            y1s = [ypool.tile([P, R], f16, tag=f"y1_{m}", name=f"y1_{m}") for m in range(KH)]
            y2s = [ypool.tile([P, R], f16, tag=f"y2_{m}", name=f"y2_{m}") for m in range(KH)]
            pred_t = ypool.tile([1, R], f32, tag="pred_t")
            pred = ypool.tile([1, R], f32, tag="pred")

            def pool_b(b):
                psums = [ps.tile([P, S], f32, tag="ps", name=f"pp{b}_{m}") for m in range(KH)]
                if b < 2:
                    # k-outer: fire all h-tiles of an arrived t-chunk at once
                    for k in range(KT):
                        for m in range(KH):
                            nc.tensor.matmul(
                                psums[m],
                                lhsT=hb_tile(b, k, m),
                                rhs=mt_tile(b, k),
                                start=(k == 0),
                                stop=(k == KT - 1),
                            )
                else:
                    # m-outer: short psum lifetimes
                    for m in range(KH):
                        for k in range(KT):
                            nc.tensor.matmul(
                                psums[m],
                                lhsT=hb_tile(b, k, m),
                                rhs=mt_tile(b, k),
                                start=(k == 0),
                                stop=(k == KT - 1),
                            )
                for m in range(KH):
                    nc.vector.tensor_copy(xts[m][:, b * S : (b + 1) * S], psums[m])

            def fc(w_sb, b_sb, xs, outs, rc, func):
                for m in range(KH):
                    pt = ps.tile([P, RC], f32, tag="ps", name=f"fc{rc}_{m}")
                    for k in range(KH):
                        nc.tensor.matmul(
                            pt,
                            lhsT=w_sb[:, k * H + m * P : k * H + (m + 1) * P],
                            rhs=xs[k][:, rc * RC : (rc + 1) * RC],
                            start=(k == 0),
                            stop=(k == KH - 1),
                        )
                    nc.scalar.activation(
                        outs[m][:, rc * RC : (rc + 1) * RC],
                        pt,
                        func,
                        bias=b_sb[:, m : m + 1],
                    )

            def fc3(rc):
                pt = ps.tile([1, RC], f32, tag="ps", name=f"fc3_{rc}")
                for k in range(KH):
                    nc.tensor.matmul(
                        pt,
                        lhsT=w3_sb[:, k : k + 1],
                        rhs=y2s[k][:, rc * RC : (rc + 1) * RC],
                        start=(k == 0),
                        stop=(k == KH - 1),
                    )
                # sigmoid(z) = 0.5 + 0.5*tanh(z/2); w3/b3 pre-scaled by 0.5
                nc.scalar.activation(
                    pred_t[:, rc * RC : (rc + 1) * RC],
                    pt,
                    FT.Tanh,
                    bias=b3_sb,
                )
                nc.vector.tensor_scalar(
                    pred[:, rc * RC : (rc + 1) * RC],
                    pred_t[:, rc * RC : (rc + 1) * RC],
                    0.5,
                    0.5,
                    OP.mult,
                    OP.add,
                )
                # stream this chunk's predictions out immediately; only the
                # final 1 KB remains on the critical path after the last tanh
                nc.sync.dma_start(
                    out.rearrange("b s -> (b s)")[rc * RC : (rc + 1) * RC],
                    pred[:, rc * RC : (rc + 1) * RC],
                )

            gelu = FT.Gelu
            pool_b(0)
            pool_b(1)
            fc(w1_sb, b1_sb, xts, y1s, 0, gelu)
            pool_b(2)
            pool_b(3)
            fc(w1_sb, b1_sb, xts, y1s, 1, gelu)
            fc(w2_sb, b2_sb, y1s, y2s, 0, gelu)
            fc3(0)
            pool_b(4)
            pool_b(5)
            fc(w1_sb, b1_sb, xts, y1s, 2, gelu)
            fc(w2_sb, b2_sb, y1s, y2s, 1, gelu)
            fc3(1)
            pool_b(6)
            pool_b(7)
            fc(w1_sb, b1_sb, xts, y1s, 3, gelu)
            fc(w2_sb, b2_sb, y1s, y2s, 2, gelu)
            fc3(2)
            fc(w2_sb, b2_sb, y1s, y2s, 3, gelu)
            fc3(3)

    nc.compile()
    return nc


def _get_program():
    if "nc" not in _CACHE:
        _CACHE["nc"] = _build_program()
    return _CACHE["nc"]


def make_in_maps(hidden, statements_ids, w1, b1, w2, b2, w3, b3):
    hidden = np.asarray(hidden, dtype=np.float32)
    sid = np.asarray(statements_ids, dtype=np.int32)

    # [B, P, KT*H] fp16, partition = token-within-tile
    hid16 = (
        hidden.astype(np.float16)
        .reshape(B, KT, P, H)
        .transpose(0, 2, 1, 3)
        .reshape(B, P, KT * H)
    )
    # count-normalized one-hot: mt[b, t, s] = (sid[b,t]==s) / max(cnt[b,s], 1)
    onehot = sid[:, :, None] == np.arange(S, dtype=np.int32)[None, None, :]
    cnt = onehot.sum(axis=1).astype(np.float32)
    inv = 1.0 / np.maximum(cnt, 1.0)
    mtp = (
        (onehot.astype(np.float32) * inv[:, None, :])
        .astype(np.float16)
        .reshape(B, KT, P, S)
        .transpose(0, 2, 1, 3)  # [B, P, KT, S]
        .reshape(B, P, KT * S)
    )

    def packw(w):
        return np.ascontiguousarray(
            np.asarray(w, np.float32)
            .astype(np.float16)
            .reshape(KH, P, H)
            .transpose(1, 0, 2)
            .reshape(P, KH * H)
        )

    w1p, w2p = packw(w1), packw(w2)
    # sigmoid-as-tanh: pre-scale the last layer by 0.5
    wpack = np.ascontiguousarray(
        (0.5 * np.asarray(w3, np.float32)).astype(np.float16).reshape(KH, P).T
    )
    cpack = np.zeros((P, 13), dtype=np.float32)
    cpack[:, 0:KH] = np.asarray(b1, np.float32).reshape(KH, P).T
    cpack[:, KH : 2 * KH] = np.asarray(b2, np.float32).reshape(KH, P).T
    cpack[0, 12] = np.float32(0.5 * np.asarray(b3).reshape(-1)[0])

    in_maps = []
    for c in range(N_CORES):
        mtp_c = (
            mtp[c * BL : (c + 1) * BL]
            .transpose(1, 0, 2)  # [P, BL, KT*S]
            .reshape(P, BL * KT * S)
        )
        in_maps.append(
            {
                "hidden": np.ascontiguousarray(hid16[c * BL : (c + 1) * BL]),
                "mtp": np.ascontiguousarray(mtp_c),
                "w1": w1p,
                "w2": w2p,
                "wpack": wpack,
                "cpack": cpack,
            }
        )
    return in_maps


def kernel(hidden, statements_ids, w1, b1, w2, b2, w3, b3, **kwargs):
    nc = _get_program()
    in_maps = make_in_maps(hidden, statements_ids, w1, b1, w2, b2, w3, b3)
    trace = bool(int(os.environ.get("KERNEL_TRACE", "0")))
    res = bass_utils.run_bass_kernel_spmd(
        nc, in_maps, core_ids=list(range(N_CORES)), trace=trace
    )
    _CACHE["last_results"] = res
    out = np.concatenate([res.results[c]["out"] for c in range(N_CORES)], axis=0)
    return out.astype(np.float32)


# revision 11
# speedup vs baseline: 1.0732x; 1.0023x over previous
"""Trainium2 Bass kernel: per-batch segment-mean pooling + 3-layer MLP.

Reference computation (B=64, T=512, H=768, S=128):
  pooled[b,s,:] = mean over t of hidden[b,t,:] where statements_ids[b,t]==s
  x = gelu(pooled @ w1 + b1); x = gelu(x @ w2 + b2)
  out[b,s] = sigmoid(x @ w3 + b3)

Distribution: data-parallel over batch across 8 NeuronCores (8 batches per
core); MLP weights replicated.

Per-core algorithm:
  - Host precomputes the count-normalized one-hot MT[t,s] = inv[s]*(sid[t]==s)
    (counts depend only on statements_ids) and ships it in fp16, along with
    fp16 hidden and weights.  fp16 keeps the PE at 1 cycle/row for any moving
    size and halves HBM traffic vs fp32; 10 mantissa bits keep the rel err
    ~1e-3 (tolerance 2e-2).
  - pooled^T tiles directly: matmul(lhsT=hidden[t,h-tile], rhs=MT[t,s])
    -> psum [128h, 128s], accumulated over the 4 t-tiles.  No PE transposes,
    no on-device one-hot build, no normalization chain: the psum already
    holds mean-pooled values in the [h, s] orientation the MLP wants.
  - psum -> SBUF copies (fp16) on DVE (GPSIMD can't read PSUM on TRN2).
  - MLP batched over all 8 local batches: rows = 8*128 = 1024 moving dim,
    weights stationary; gelu + bias fused on ACT.
  - sigmoid(z) = 0.5 + 0.5*tanh(z/2): tanh lives in the same ACT function
    table as gelu, so the 8 x 1.28us ACT_TABLE_LOAD thrash of alternating
    gelu/sigmoid tables disappears.  w3/b3 are pre-scaled by 0.5 on the
    host; the final 0.5*t+0.5 affine runs on the otherwise-idle DVE.
  - DMA issue (~600ns per descriptor on an engine's HWDGE queue) is split:
    weights on the ACT queue, the hidden/MT stream on sync, so neither
    stream's issue latency delays the other's arrival.
"""

import os
import sys

sys.path.insert(0, "/opt/trn_rl_repo")

import numpy as np

import concourse.bass as bass
import concourse.mybir as mybir
import concourse.tile as tile
from concourse import bacc, bass_utils

B, T, H, S = 64, 512, 768, 128
N_CORES = 8
BL = B // N_CORES  # local batches per core
P = 128
KT = T // P        # t-tiles per batch
KH = H // P        # h-tiles
R = BL * S         # MLP rows per core
RC = 2 * S         # moving-dim chunk (2 batches)
NRC = R // RC

_CACHE: dict = {}


def _build_program():
    f32, f16 = mybir.dt.float32, mybir.dt.float16
    FT = mybir.ActivationFunctionType
    OP = mybir.AluOpType

    nc = bacc.Bacc("TRN2", target_bir_lowering=False, debug=False)
    hid = nc.dram_tensor("hidden", [BL, P, KT * H], f16, kind="ExternalInput").ap()
    mtp = nc.dram_tensor("mtp", [P, BL * KT * S], f16, kind="ExternalInput").ap()
    w1 = nc.dram_tensor("w1", [P, KH * H], f16, kind="ExternalInput").ap()
    w2 = nc.dram_tensor("w2", [P, KH * H], f16, kind="ExternalInput").ap()
    wpack = nc.dram_tensor("wpack", [P, KH], f16, kind="ExternalInput").ap()
    cpack = nc.dram_tensor("cpack", [P, 13], f32, kind="ExternalInput").ap()
    out = nc.dram_tensor("out", [BL, S], f32, kind="ExternalOutput").ap()

    with tile.TileContext(nc) as tc:
        with (
            tc.tile_pool(name="consts", bufs=1) as consts,
            tc.tile_pool(name="wpool", bufs=1) as wpool,
            tc.tile_pool(name="hpool", bufs=1) as hpool,
            tc.tile_pool(name="xtpool", bufs=1) as xtpool,
            tc.tile_pool(name="ypool", bufs=1) as ypool,
            tc.tile_pool(name="ps", bufs=8, space="PSUM") as ps,
        ):
            # Two HWDGE queues (sync + ACT) transfer in parallel; each queue
            # is serial in issue order, so the streams are interleaved to
            # match compute-consumption order on both queues.
            cpf_sb = consts.tile([P, 13], f32)
            nc.scalar.dma_start(cpf_sb, cpack)
            w3_sb = consts.tile([P, KH], f16, name="w3_sb")
            nc.scalar.dma_start(w3_sb, wpack)
            b1_sb = cpf_sb[:, 0:KH]
            b2_sb = cpf_sb[:, KH : 2 * KH]
            b3_sb = cpf_sb[0:1, 12:13]

            mt_sb = hpool.tile([P, BL * KT * S], f16, name="mt_sb")
            hbs = [None] * BL

            def load_mt(eng, b0, b1):
                eng.dma_start(
                    mt_sb[:, b0 * KT * S : b1 * KT * S],
                    mtp[:, b0 * KT * S : b1 * KT * S],
                )

            def load_hb(eng, b):
                if b < 2:
                    # first batches arrive per t-tile so pooling starts on the
                    # first 0.2 MB instead of the full 0.8 MB batch
                    tiles = []
                    for k in range(KT):
                        t = hpool.tile([P, H], f16, tag=f"hb{b}k{k}", name=f"hb{b}k{k}")
                        eng.dma_start(t, hid[b][:, k * H : (k + 1) * H])
                        tiles.append(t)
                    hbs[b] = tiles
                else:
                    hb = hpool.tile([P, KT * H], f16, tag=f"hb{b}", name=f"hb{b}")
                    eng.dma_start(hb, hid[b])
                    hbs[b] = hb

            def hb_tile(b, k, m):
                if b < 2:
                    return hbs[b][k][:, m * P : (m + 1) * P]
                return hbs[b][:, k * H + m * P : k * H + (m + 1) * P]

            def mt_tile(b, k):
                return mt_sb[:, (b * KT + k) * S : (b * KT + k + 1) * S]

            # sync: the batches the pooling front consumes first (empirically
            # the single-queue layout beats every gpsimd/scalar spill tried)
            load_mt(nc.sync, 0, 1)
            load_hb(nc.sync, 0)
            load_mt(nc.sync, 1, 2)
            load_hb(nc.sync, 1)
            load_mt(nc.sync, 2, BL)
            for b in range(2, BL):
                load_hb(nc.sync, b)
            # ACT queue: weights only — a 5th DMA here could hit semaphore
            # rotation and stall the in-order ACT engine's gelus behind it
            w1_sb = wpool.tile([P, KH * H], f16, name="w1_sb")
            nc.scalar.dma_start(w1_sb, w1)
            w2_sb = wpool.tile([P, KH * H], f16, name="w2_sb")
            nc.scalar.dma_start(w2_sb, w2)

            xts = [xtpool.tile([P, R], f16, tag=f"xt{k}", name=f"xt{k}") for k in range(KH)]
            y1s = [ypool.tile([P, R], f16, tag=f"y1_{m}", name=f"y1_{m}") for m in range(KH)]
            y2s = [ypool.tile([P, R], f16, tag=f"y2_{m}", name=f"y2_{m}") for m in range(KH)]
            pred_t = ypool.tile([1, R], f32, tag="pred_t")
            pred = ypool.tile([1, R], f32, tag="pred")

            def pool_b(b):
                psums = [ps.tile([P, S], f32, tag="ps", name=f"pp{b}_{m}") for m in range(KH)]
                if b < 2:
                    # k-outer: fire all h-tiles of an arrived t-chunk at once
                    for k in range(KT):
                        for m in range(KH):
                            nc.tensor.matmul(
                                psums[m],
                                lhsT=hb_tile(b, k, m),
                                rhs=mt_tile(b, k),
                                start=(k == 0),
                                stop=(k == KT - 1),
                            )
                else:
                    # m-outer: short psum lifetimes
                    for m in range(KH):
                        for k in range(KT):
                            nc.tensor.matmul(
                                psums[m],
                                lhsT=hb_tile(b, k, m),
                                rhs=mt_tile(b, k),
                                start=(k == 0),
                                stop=(k == KT - 1),
                            )
                for m in range(KH):
                    nc.vector.tensor_copy(xts[m][:, b * S : (b + 1) * S], psums[m])

            def fc(w_sb, b_sb, xs, outs, lo, hi, func):
                for m in range(KH):
                    pt = ps.tile([P, hi - lo], f32, tag="ps", name=f"fc{lo}_{m}")
                    for k in range(KH):
                        nc.tensor.matmul(
                            pt,
                            lhsT=w_sb[:, k * H + m * P : k * H + (m + 1) * P],
                            rhs=xs[k][:, lo:hi],
                            start=(k == 0),
                            stop=(k == KH - 1),
                        )
                    nc.scalar.activation(
                        outs[m][:, lo:hi],
                        pt,
                        func,
                        bias=b_sb[:, m : m + 1],
                    )

            def fc3(lo, hi):
                pt = ps.tile([1, hi - lo], f32, tag="ps", name=f"fc3_{lo}")
                for k in range(KH):
                    nc.tensor.matmul(
                        pt,
                        lhsT=w3_sb[:, k : k + 1],
                        rhs=y2s[k][:, lo:hi],
                        start=(k == 0),
                        stop=(k == KH - 1),
                    )
                # sigmoid(z) = 0.5 + 0.5*tanh(z/2); w3/b3 pre-scaled by 0.5
                nc.scalar.activation(
                    pred_t[:, lo:hi],
                    pt,
                    FT.Tanh,
                    bias=b3_sb,
                )
                nc.vector.tensor_scalar(
                    pred[:, lo:hi],
                    pred_t[:, lo:hi],
                    0.5,
                    0.5,
                    OP.mult,
                    OP.add,
                )
                # stream this chunk's predictions out immediately; only the
                # final 1 KB remains on the critical path after the last tanh
                nc.sync.dma_start(
                    out.rearrange("b s -> (b s)")[lo:hi],
                    pred[:, lo:hi],
                )

            gelu = FT.Gelu
            pool_b(0)
            # single-batch fc1 chunk fills the PE hole while hid1 streams in
            fc(w1_sb, b1_sb, xts, y1s, 0, S, gelu)
            pool_b(1)
            fc(w1_sb, b1_sb, xts, y1s, S, 2 * S, gelu)
            pool_b(2)
            pool_b(3)
            fc(w1_sb, b1_sb, xts, y1s, 2 * S, 4 * S, gelu)
            fc(w2_sb, b2_sb, y1s, y2s, 0, 2 * S, gelu)
            fc3(0, 2 * S)
            pool_b(4)
            pool_b(5)
            fc(w1_sb, b1_sb, xts, y1s, 4 * S, 6 * S, gelu)
            fc(w2_sb, b2_sb, y1s, y2s, 2 * S, 4 * S, gelu)
            fc3(2 * S, 4 * S)
            pool_b(6)
            pool_b(7)
            fc(w1_sb, b1_sb, xts, y1s, 6 * S, 8 * S, gelu)
            fc(w2_sb, b2_sb, y1s, y2s, 4 * S, 6 * S, gelu)
            fc3(4 * S, 6 * S)
            fc(w2_sb, b2_sb, y1s, y2s, 6 * S, 8 * S, gelu)
            fc3(6 * S, 8 * S)

    nc.compile()
    return nc


def _get_program():
    if "nc" not in _CACHE:
        _CACHE["nc"] = _build_program()
    return _CACHE["nc"]


def make_in_maps(hidden, statements_ids, w1, b1, w2, b2, w3, b3):
    hidden = np.asarray(hidden, dtype=np.float32)
    sid = np.asarray(statements_ids, dtype=np.int32)

    # [B, P, KT*H] fp16, partition = token-within-tile
    hid16 = (
        hidden.astype(np.float16)
        .reshape(B, KT, P, H)
        .transpose(0, 2, 1, 3)
        .reshape(B, P, KT * H)
    )
    # count-normalized one-hot: mt[b, t, s] = (sid[b,t]==s) / max(cnt[b,s], 1)
    onehot = sid[:, :, None] == np.arange(S, dtype=np.int32)[None, None, :]
    cnt = onehot.sum(axis=1).astype(np.float32)
    inv = 1.0 / np.maximum(cnt, 1.0)
    mtp = (
        (onehot.astype(np.float32) * inv[:, None, :])
        .astype(np.float16)
        .reshape(B, KT, P, S)
        .transpose(0, 2, 1, 3)  # [B, P, KT, S]
        .reshape(B, P, KT * S)
    )

    def packw(w):
        return np.ascontiguousarray(
            np.asarray(w, np.float32)
            .astype(np.float16)
            .reshape(KH, P, H)
            .transpose(1, 0, 2)
            .reshape(P, KH * H)
        )

    w1p, w2p = packw(w1), packw(w2)
    # sigmoid-as-tanh: pre-scale the last layer by 0.5
    wpack = np.ascontiguousarray(
        (0.5 * np.asarray(w3, np.float32)).astype(np.float16).reshape(KH, P).T
    )
    cpack = np.zeros((P, 13), dtype=np.float32)
    cpack[:, 0:KH] = np.asarray(b1, np.float32).reshape(KH, P).T
    cpack[:, KH : 2 * KH] = np.asarray(b2, np.float32).reshape(KH, P).T
    cpack[0, 12] = np.float32(0.5 * np.asarray(b3).reshape(-1)[0])

    in_maps = []
    for c in range(N_CORES):
        mtp_c = (
            mtp[c * BL : (c + 1) * BL]
            .transpose(1, 0, 2)  # [P, BL, KT*S]
            .reshape(P, BL * KT * S)
        )
        in_maps.append(
            {
                "hidden": np.ascontiguousarray(hid16[c * BL : (c + 1) * BL]),
                "mtp": np.ascontiguousarray(mtp_c),
                "w1": w1p,
                "w2": w2p,
                "wpack": wpack,
                "cpack": cpack,
            }
        )
    return in_maps


def kernel(hidden, statements_ids, w1, b1, w2, b2, w3, b3, **kwargs):
    nc = _get_program()
    in_maps = make_in_maps(hidden, statements_ids, w1, b1, w2, b2, w3, b3)
    trace = bool(int(os.environ.get("KERNEL_TRACE", "0")))
    res = bass_utils.run_bass_kernel_spmd(
        nc, in_maps, core_ids=list(range(N_CORES)), trace=trace
    )
    _CACHE["last_results"] = res
    out = np.concatenate([res.results[c]["out"] for c in range(N_CORES)], axis=0)
    return out.astype(np.float32)


# revision 12
# speedup vs baseline: 1.0877x; 1.0135x over previous
"""Trainium2 Bass kernel: per-batch segment-mean pooling + 3-layer MLP.

Reference computation (B=64, T=512, H=768, S=128):
  pooled[b,s,:] = mean over t of hidden[b,t,:] where statements_ids[b,t]==s
  x = gelu(pooled @ w1 + b1); x = gelu(x @ w2 + b2)
  out[b,s] = sigmoid(x @ w3 + b3)

Distribution: data-parallel over batch across 8 NeuronCores (8 batches per
core); MLP weights replicated.

Per-core algorithm:
  - Host precomputes the count-normalized one-hot MT[t,s] = inv[s]*(sid[t]==s)
    (counts depend only on statements_ids) and ships it in fp16, along with
    fp16 hidden and weights.  fp16 keeps the PE at 1 cycle/row for any moving
    size and halves HBM traffic vs fp32; 10 mantissa bits keep the rel err
    ~1e-3 (tolerance 2e-2).
  - pooled^T tiles directly: matmul(lhsT=hidden[t,h-tile], rhs=MT[t,s])
    -> psum [128h, 128s], accumulated over the 4 t-tiles.  No PE transposes,
    no on-device one-hot build, no normalization chain: the psum already
    holds mean-pooled values in the [h, s] orientation the MLP wants.
  - psum -> SBUF copies (fp16) on DVE (GPSIMD can't read PSUM on TRN2).
  - MLP batched over all 8 local batches: rows = 8*128 = 1024 moving dim,
    weights stationary; gelu + bias fused on ACT.
  - sigmoid(z) = 0.5 + 0.5*tanh(z/2): tanh lives in the same ACT function
    table as gelu, so the 8 x 1.28us ACT_TABLE_LOAD thrash of alternating
    gelu/sigmoid tables disappears.  w3/b3 are pre-scaled by 0.5 on the
    host; the final 0.5*t+0.5 affine runs on the otherwise-idle DVE.
  - DMA issue (~600ns per descriptor on an engine's HWDGE queue) is split:
    weights on the ACT queue, the hidden/MT stream on sync, so neither
    stream's issue latency delays the other's arrival.
"""

import os
import sys

sys.path.insert(0, "/opt/trn_rl_repo")

import numpy as np

import concourse.bass as bass
import concourse.mybir as mybir
import concourse.tile as tile
from concourse import bacc, bass_utils

B, T, H, S = 64, 512, 768, 128
N_CORES = 8
BL = B // N_CORES  # local batches per core
P = 128
KT = T // P        # t-tiles per batch
KH = H // P        # h-tiles
R = BL * S         # MLP rows per core
RC = 2 * S         # moving-dim chunk (2 batches)
NRC = R // RC

_CACHE: dict = {}


def _build_program():
    f32, f16 = mybir.dt.float32, mybir.dt.float16
    FT = mybir.ActivationFunctionType
    OP = mybir.AluOpType

    nc = bacc.Bacc("TRN2", target_bir_lowering=False, debug=False)
    hid = nc.dram_tensor("hidden", [BL, P, KT * H], f16, kind="ExternalInput").ap()
    mtp = nc.dram_tensor("mtp", [P, BL * KT * S], f16, kind="ExternalInput").ap()
    w1 = nc.dram_tensor("w1", [P, KH * H], f16, kind="ExternalInput").ap()
    w2 = nc.dram_tensor("w2", [P, KH * H], f16, kind="ExternalInput").ap()
    wpack = nc.dram_tensor("wpack", [P, KH], f16, kind="ExternalInput").ap()
    cpack = nc.dram_tensor("cpack", [P, 13], f32, kind="ExternalInput").ap()
    out = nc.dram_tensor("out", [BL, S], f32, kind="ExternalOutput").ap()

    with tile.TileContext(nc) as tc:
        with (
            tc.tile_pool(name="consts", bufs=1) as consts,
            tc.tile_pool(name="wpool", bufs=1) as wpool,
            tc.tile_pool(name="hpool", bufs=1) as hpool,
            tc.tile_pool(name="xtpool", bufs=1) as xtpool,
            tc.tile_pool(name="ypool", bufs=1) as ypool,
            tc.tile_pool(name="ps", bufs=8, space="PSUM") as ps,
        ):
            # Two HWDGE queues (sync + ACT) transfer in parallel; each queue
            # is serial in issue order, so the streams are interleaved to
            # match compute-consumption order on both queues.
            cpf_sb = consts.tile([P, 13], f32)
            nc.scalar.dma_start(cpf_sb, cpack)
            w3_sb = consts.tile([P, KH], f16, name="w3_sb")
            nc.scalar.dma_start(w3_sb, wpack)
            b1_sb = cpf_sb[:, 0:KH]
            b2_sb = cpf_sb[:, KH : 2 * KH]
            b3_sb = cpf_sb[0:1, 12:13]

            mt_sb = hpool.tile([P, BL * KT * S], f16, name="mt_sb")
            hbs = [None] * BL

            def load_mt(eng, b0, b1):
                eng.dma_start(
                    mt_sb[:, b0 * KT * S : b1 * KT * S],
                    mtp[:, b0 * KT * S : b1 * KT * S],
                )

            def load_hb(eng, b):
                if b < 2:
                    # first batches arrive per t-tile so pooling starts on the
                    # first 0.2 MB instead of the full 0.8 MB batch
                    tiles = []
                    for k in range(KT):
                        t = hpool.tile([P, H], f16, tag=f"hb{b}k{k}", name=f"hb{b}k{k}")
                        eng.dma_start(t, hid[b][:, k * H : (k + 1) * H])
                        tiles.append(t)
                    hbs[b] = tiles
                else:
                    hb = hpool.tile([P, KT * H], f16, tag=f"hb{b}", name=f"hb{b}")
                    eng.dma_start(hb, hid[b])
                    hbs[b] = hb

            def hb_tile(b, k, m):
                if b < 2:
                    return hbs[b][k][:, m * P : (m + 1) * P]
                return hbs[b][:, k * H + m * P : k * H + (m + 1) * P]

            def mt_tile(b, k):
                return mt_sb[:, (b * KT + k) * S : (b * KT + k + 1) * S]

            # sync: the batches the pooling front consumes first (empirically
            # the single-queue layout beats every gpsimd/scalar spill tried)
            load_mt(nc.sync, 0, 1)
            load_hb(nc.sync, 0)
            load_mt(nc.sync, 1, 2)
            load_hb(nc.sync, 1)
            load_mt(nc.sync, 2, BL)
            for b in range(2, BL):
                load_hb(nc.sync, b)
            # ACT queue: weights only — a 5th DMA here could hit semaphore
            # rotation and stall the in-order ACT engine's gelus behind it
            w1_sb = wpool.tile([P, KH * H], f16, name="w1_sb")
            nc.scalar.dma_start(w1_sb, w1)
            w2_sb = wpool.tile([P, KH * H], f16, name="w2_sb")
            nc.scalar.dma_start(w2_sb, w2)

            xts = [xtpool.tile([P, R], f16, tag=f"xt{k}", name=f"xt{k}") for k in range(KH)]
            y1s = [ypool.tile([P, R], f16, tag=f"y1_{m}", name=f"y1_{m}") for m in range(KH)]
            y2s = [ypool.tile([P, R], f16, tag=f"y2_{m}", name=f"y2_{m}") for m in range(KH)]
            pred_t = ypool.tile([1, R], f32, tag="pred_t")
            pred = ypool.tile([1, R], f32, tag="pred")

            def pool_b(b):
                psums = [ps.tile([P, S], f32, tag="ps", name=f"pp{b}_{m}") for m in range(KH)]
                if b < 2:
                    # k-outer: fire all h-tiles of an arrived t-chunk at once
                    for k in range(KT):
                        for m in range(KH):
                            nc.tensor.matmul(
                                psums[m],
                                lhsT=hb_tile(b, k, m),
                                rhs=mt_tile(b, k),
                                start=(k == 0),
                                stop=(k == KT - 1),
                            )
                else:
                    # m-outer: short psum lifetimes
                    for m in range(KH):
                        for k in range(KT):
                            nc.tensor.matmul(
                                psums[m],
                                lhsT=hb_tile(b, k, m),
                                rhs=mt_tile(b, k),
                                start=(k == 0),
                                stop=(k == KT - 1),
                            )
                for m in range(KH):
                    nc.vector.tensor_copy(xts[m][:, b * S : (b + 1) * S], psums[m])

            def fc(w_sb, b_sb, xs, outs, lo, hi, func):
                for m in range(KH):
                    pt = ps.tile([P, hi - lo], f32, tag="ps", name=f"fc{lo}_{m}")
                    for k in range(KH):
                        nc.tensor.matmul(
                            pt,
                            lhsT=w_sb[:, k * H + m * P : k * H + (m + 1) * P],
                            rhs=xs[k][:, lo:hi],
                            start=(k == 0),
                            stop=(k == KH - 1),
                        )
                    nc.scalar.activation(
                        outs[m][:, lo:hi],
                        pt,
                        func,
                        bias=b_sb[:, m : m + 1],
                    )

            def fc3(lo, hi):
                pt = ps.tile([1, hi - lo], f32, tag="ps", name=f"fc3_{lo}")
                for k in range(KH):
                    nc.tensor.matmul(
                        pt,
                        lhsT=w3_sb[:, k : k + 1],
                        rhs=y2s[k][:, lo:hi],
                        start=(k == 0),
                        stop=(k == KH - 1),
                    )
                # sigmoid(z) = 0.5 + 0.5*tanh(z/2); w3/b3 pre-scaled by 0.5
                nc.scalar.activation(
                    pred_t[:, lo:hi],
                    pt,
                    FT.Tanh,
                    bias=b3_sb,
                )
                nc.vector.tensor_scalar(
                    pred[:, lo:hi],
                    pred_t[:, lo:hi],
                    0.5,
                    0.5,
                    OP.mult,
                    OP.add,
                )
                # stream this chunk's predictions out immediately; only the
                # final 1 KB remains on the critical path after the last tanh
                nc.sync.dma_start(
                    out.rearrange("b s -> (b s)")[lo:hi],
                    pred[:, lo:hi],
                )

            gelu = FT.Gelu
            pool_b(0)
            pool_b(1)
            fc(w1_sb, b1_sb, xts, y1s, 0, 2 * S, gelu)
            pool_b(2)
            pool_b(3)
            fc(w1_sb, b1_sb, xts, y1s, 2 * S, 4 * S, gelu)
            fc(w2_sb, b2_sb, y1s, y2s, 0, 2 * S, gelu)
            fc3(0, 2 * S)
            pool_b(4)
            pool_b(5)
            fc(w1_sb, b1_sb, xts, y1s, 4 * S, 6 * S, gelu)
            fc(w2_sb, b2_sb, y1s, y2s, 2 * S, 4 * S, gelu)
            fc3(2 * S, 4 * S)
            pool_b(6)
            pool_b(7)
            fc(w1_sb, b1_sb, xts, y1s, 6 * S, 8 * S, gelu)
            fc(w2_sb, b2_sb, y1s, y2s, 4 * S, 6 * S, gelu)
            fc3(4 * S, 6 * S)
            fc(w2_sb, b2_sb, y1s, y2s, 6 * S, 8 * S, gelu)
            fc3(6 * S, 8 * S)

    nc.compile()
    return nc


def _get_program():
    if "nc" not in _CACHE:
        _CACHE["nc"] = _build_program()
    return _CACHE["nc"]


def make_in_maps(hidden, statements_ids, w1, b1, w2, b2, w3, b3):
    hidden = np.asarray(hidden, dtype=np.float32)
    sid = np.asarray(statements_ids, dtype=np.int32)

    # [B, P, KT*H] fp16, partition = token-within-tile
    hid16 = (
        hidden.astype(np.float16)
        .reshape(B, KT, P, H)
        .transpose(0, 2, 1, 3)
        .reshape(B, P, KT * H)
    )
    # count-normalized one-hot: mt[b, t, s] = (sid[b,t]==s) / max(cnt[b,s], 1)
    onehot = sid[:, :, None] == np.arange(S, dtype=np.int32)[None, None, :]
    cnt = onehot.sum(axis=1).astype(np.float32)
    inv = 1.0 / np.maximum(cnt, 1.0)
    mtp = (
        (onehot.astype(np.float32) * inv[:, None, :])
        .astype(np.float16)
        .reshape(B, KT, P, S)
        .transpose(0, 2, 1, 3)  # [B, P, KT, S]
        .reshape(B, P, KT * S)
    )

    def packw(w):
        return np.ascontiguousarray(
            np.asarray(w, np.float32)
            .astype(np.float16)
            .reshape(KH, P, H)
            .transpose(1, 0, 2)
            .reshape(P, KH * H)
        )

    w1p, w2p = packw(w1), packw(w2)
    # sigmoid-as-tanh: pre-scale the last layer by 0.5
    wpack = np.ascontiguousarray(
        (0.5 * np.asarray(w3, np.float32)).astype(np.float16).reshape(KH, P).T
    )
    cpack = np.zeros((P, 13), dtype=np.float32)
    cpack[:, 0:KH] = np.asarray(b1, np.float32).reshape(KH, P).T
    cpack[:, KH : 2 * KH] = np.asarray(b2, np.float32).reshape(KH, P).T
    cpack[0, 12] = np.float32(0.5 * np.asarray(b3).reshape(-1)[0])

    in_maps = []
    for c in range(N_CORES):
        mtp_c = (
            mtp[c * BL : (c + 1) * BL]
            .transpose(1, 0, 2)  # [P, BL, KT*S]
            .reshape(P, BL * KT * S)
        )
        in_maps.append(
            {
                "hidden": np.ascontiguousarray(hid16[c * BL : (c + 1) * BL]),
                "mtp": np.ascontiguousarray(mtp_c),
                "w1": w1p,
                "w2": w2p,
                "wpack": wpack,
                "cpack": cpack,
            }
        )
    return in_maps


def kernel(hidden, statements_ids, w1, b1, w2, b2, w3, b3, **kwargs):
    nc = _get_program()
    in_maps = make_in_maps(hidden, statements_ids, w1, b1, w2, b2, w3, b3)
    trace = bool(int(os.environ.get("KERNEL_TRACE", "0")))
    res = bass_utils.run_bass_kernel_spmd(
        nc, in_maps, core_ids=list(range(N_CORES)), trace=trace
    )
    _CACHE["last_results"] = res
    out = np.concatenate([res.results[c]["out"] for c in range(N_CORES)], axis=0)
    return out.astype(np.float32)
